# revision 10
# baseline (speedup 1.0000x reference)
"""GCN (2x GCNConv + linear head) on 8 Trainium2 NeuronCores.

Strategy (graph/data parallel per the node-sharding hint):
  - dst nodes sharded across 8 cores (12500 each); 128x128 weights replicated.
  - Symmetric normalization folded into table rows: table[n] = dinv[n]*(x@W)[n];
    aggregate unweighted (self-loop = just another message), multiply by
    dinv[dst] afterwards.
  - Per layer: matmul the core's table shard -> AllGather the full
    [100352,128] message table (with a random graph every core needs nearly
    every node, so full replication beats point-to-point halos) -> bulk
    gather of per-edge messages with `dma_gather` -> segmented reduction.
  - dma_gather has int16 indices, so the table is read in 4 chunks of 25088
    rows. Messages are aggregated in TWO levels: level 1 reduces each dst's
    messages *within a chunk* on a per-chunk degree-sorted grid (sorting
    keeps region widths uniform -> little padding), writing one partial row
    per (dst, chunk) to DRAM; level 2 gathers each dst's 4 partials (uniform
    width -> trivially regular) in canonical dst order and applies
    dinv/bias/relu.
"""

import numpy as np

import concourse.bacc as bacc
import concourse.bass as bass
import concourse.mybir as mybir
import concourse.tile as tile
from concourse import library_config
from concourse.bass_utils import run_bass_kernel_spmd
from concourse.masks import make_identity

N_NODES = 100000
N_CORES = 8
D = 128
P = 128

F32 = mybir.dt.float32
I16 = mybir.dt.int16
AX = mybir.AxisListType
ALU = mybir.AluOpType
ACTF = mybir.ActivationFunctionType

GMAX = 32          # max grid columns per message buffer granule
CALL_COLS = 8      # max grid columns per dma_gather call: single_packet
                   # SWDGE = 64 descriptors x 16 engines = 1024 indices
                   # (1536 was observed to DMA-abort on HW)


def _pack_idx(arr2d):
    """[cols, 128] int array (position i = col*128+p -> arr2d[col, p]) to the
    dma_gather idx tile layout: wrap by 16, replicate to 128 partitions."""
    flat = arr2d.reshape(-1)
    assert flat.size % 16 == 0
    wrapped = flat.reshape(-1, 16).T.astype(np.int16)  # [16, n/16]
    return np.tile(wrapped, (8, 1))  # [128, n/16]


def _granules(widths, gmax):
    """Split region widths into granules of consecutive regions with total
    width <= gmax. Returns (first_region, n_regions, col0, wsum)."""
    out = []
    r0 = 0
    R = len(widths)
    coloff = np.concatenate([[0], np.cumsum(widths)]).astype(int)
    while r0 < R:
        tot = int(widths[r0])
        r1 = r0 + 1
        while r1 < R and tot + int(widths[r1]) <= gmax:
            tot += int(widths[r1])
            r1 += 1
        out.append((r0, r1 - r0, int(coloff[r0]), tot))
        r0 = r1
    return out


def _plan(edge_index, n_nodes, n_cores):
    src = edge_index[0].astype(np.int64)
    dst = edge_index[1].astype(np.int64)
    E = src.shape[0]
    shard = n_nodes // n_cores
    R = -(-shard // P)
    rows = R * P
    assert rows > shard, "need dummy rows for zero padding"
    CH = 2 * rows                      # chunk = 2 core shards
    n_chunks = (rows * n_cores) // CH
    assert n_cores % 2 == 0 and n_chunks * CH == rows * n_cores
    assert CH - 1 <= 32767, "chunk must be int16-addressable"

    deg = np.bincount(dst, minlength=n_nodes).astype(np.int64) + 1
    dinv = (1.0 / np.sqrt(deg.astype(np.float64))).astype(np.float32)

    core_of = np.arange(n_nodes) // shard
    pos = core_of * rows + (np.arange(n_nodes) - core_of * shard)

    # all messages: edges + self-loops, as (dst, src_table_pos)
    MD = np.concatenate([dst, np.arange(n_nodes)])
    MS = np.concatenate([pos[src], pos])
    MC = MS // CH                                     # chunk of each message
    NM = MD.shape[0]
    dslot = MD - core_of[MD] * shard                  # dst local slot (<shard)
    dcore = core_of[MD]

    # per (core, slot, chunk) counts
    cnt = np.zeros((n_cores, rows, n_chunks), np.int64)
    np.add.at(cnt, (dcore, dslot, MC), 1)

    # per-chunk degree-sorted grids
    rank = np.zeros((n_cores, rows, n_chunks), np.int64)
    W = np.zeros((n_chunks, R), np.int64)
    for c in range(n_chunks):
        for k in range(n_cores):
            order = np.argsort(-cnt[k, :, c], kind="stable")
            rank[k, order, c] = np.arange(rows)
        sorted_cnt = -np.sort(-cnt[:, :, c], axis=1)       # desc per core
        W[c] = np.maximum(sorted_cnt[:, ::P].max(axis=0), 1)
    assert W.max() <= GMAX, f"region width {W.max()} > GMAX"
    coloff = np.concatenate([np.zeros((n_chunks, 1), np.int64),
                             np.cumsum(W, axis=1)], axis=1)  # [n_chunks, R+1]
    chunk_cols = coloff[:, -1]
    grid_col0 = np.concatenate([[0], np.cumsum(chunk_cols)])
    tot_cols = int(grid_col0[-1])

    # fill grids [n_cores, tot_cols, 128] with chunk-local idx; pad points at
    # a zero table row (each chunk = 2 core shards; the first shard's dummy
    # rows start at chunk-local `shard`)
    grid = np.full((n_cores, tot_cols, P), shard, np.int16)
    rk = rank[dcore, dslot, MC]
    r_m = rk // P
    p_m = rk % P
    # j-within-(dst,chunk)
    key = (dcore * rows + dslot) * n_chunks + MC
    morder = np.argsort(key, kind="stable")
    ks = key[morder]
    seg_first = np.ones(NM, bool)
    seg_first[1:] = ks[1:] != ks[:-1]
    seg_start_pos = np.where(seg_first)[0]
    seg_id = np.cumsum(seg_first) - 1
    j_sorted = np.arange(NM) - seg_start_pos[seg_id]
    j = np.empty(NM, np.int64)
    j[morder] = j_sorted
    col = grid_col0[MC] + coloff[MC, r_m] + j
    grid[dcore, col, p_m] = (MS - MC * CH).astype(np.int16)

    # level-1 partial row of (core, slot, chunk): (c*R + r)*128 + p
    prow = (rank // P + np.arange(n_chunks)[None, None, :] * R) * P + rank % P

    # level-2 grids: half A = chunks 0,1 ; half B = chunks 2,3
    half_rows = 2 * R * P
    l2 = np.empty((n_cores, 2, 2 * R, P), np.int16)
    s_all = np.arange(rows)
    p2 = s_all % P
    r2 = s_all // P
    for h in range(2):
        for t in range(2):
            c = 2 * h + t
            v = prow[:, :, c] - h * half_rows
            assert v.min() >= 0 and v.max() < half_rows
            l2[:, h, 2 * r2 + t, p2] = v.astype(np.int16)

    # granule schedules (uniform across cores)
    call_meta = []
    off = 0
    for c in range(n_chunks):
        for (gr0, gnr, gc0, gw) in _granules(W[c], GMAX):
            regs = [(int(gr0 + i), int(coloff[c, gr0 + i] - gc0),
                     int(W[c, gr0 + i])) for i in range(gnr)]
            call_meta.append(("L1", c, gc0, gw, regs, off))
            off += 8 * gw
    l2_gran = _granules([2] * R, GMAX)
    for h in range(2):
        for (gr0, gnr, gc0, gw) in l2_gran:
            call_meta.append(("L2", h, gc0, gw, gr0, gnr, off))
            off += 8 * gw
    idx_cols = off

    idx_all = np.empty((n_cores, P, idx_cols), np.int16)
    for k in range(n_cores):
        parts = []
        for meta in call_meta:
            if meta[0] == "L1":
                (_, c, gc0, gw, regs, ioff) = meta
                a = grid[k, grid_col0[c] + gc0: grid_col0[c] + gc0 + gw]
            else:
                (_, h, gc0, gw, gr0, gnr, ioff) = meta
                a = l2[k, h, gc0:gc0 + gw]
            parts.append(_pack_idx(a))
        idx_all[k] = np.concatenate(parts, axis=1)

    dinv_rows = np.zeros((n_cores, P, R), np.float32)
    sl = np.arange(shard)
    for k in range(n_cores):
        dinv_rows[k, sl % P, sl // P] = dinv[k * shard:(k + 1) * shard]

    plan = dict(shard=shard, R=R, rows=rows, CH=CH, n_chunks=n_chunks,
                table_rows=rows * n_cores, half_rows=half_rows,
                call_meta=call_meta, idx_cols=idx_cols, tot_cols=tot_cols)
    return plan, idx_all, dinv_rows


def _build_program(plan, n_cores):
    R = plan["R"]
    rows = plan["rows"]
    CH = plan["CH"]
    table_rows = plan["table_rows"]
    half_rows = plan["half_rows"]
    call_meta = plan["call_meta"]
    idx_cols = plan["idx_cols"]

    nc = bacc.Bacc("TRN2", target_bir_lowering=False, debug=False,
                   enable_asserts=False, num_devices=n_cores)

    x_t = nc.dram_tensor("x_in", [rows, D], F32, kind="ExternalInput")
    w1_t = nc.dram_tensor("W1", [D, D], F32, kind="ExternalInput")
    w2_t = nc.dram_tensor("W2", [D, D], F32, kind="ExternalInput")
    wf_t = nc.dram_tensor("Wf", [1, D], F32, kind="ExternalInput")
    b1_t = nc.dram_tensor("b1", [1, D], F32, kind="ExternalInput")
    b2_t = nc.dram_tensor("b2", [1, D], F32, kind="ExternalInput")
    bf_t = nc.dram_tensor("bf", [1, 1], F32, kind="ExternalInput")
    idx_t = nc.dram_tensor("idx", [P, idx_cols], I16, kind="ExternalInput")
    dinv_t = nc.dram_tensor("dinv_rows", [P, R], F32, kind="ExternalInput")
    out_t = nc.dram_tensor("out", [P, R], F32, kind="ExternalOutput")

    rg = [list(range(n_cores))]

    with tile.TileContext(nc) as tc:
        with (
            tc.tile_pool(name="dram", bufs=1, space="DRAM") as dpool,
            tc.tile_pool(name="const", bufs=1) as cpool,
            tc.tile_pool(name="sb", bufs=2) as spool,
            tc.tile_pool(name="ps", bufs=2, space="PSUM") as ppool,
        ):
            nc.gpsimd.load_library(library_config.mlp)

            ag_in = [dpool.tile([rows, D], F32, name=f"ag_in{i}")
                     for i in range(2)]
            table = [dpool.tile([table_rows, D], F32, name=f"table{i}")
                     for i in range(2)]
            partials = [dpool.tile([2 * half_rows, D], F32, name=f"partials{i}")
                        for i in range(2)]

            # ---- constants ----
            ident = cpool.tile([P, P], F32)
            make_identity(nc, ident[:])
            w1s = cpool.tile([D, D], F32)
            nc.sync.dma_start(out=w1s[:], in_=w1_t.ap())
            w2s = cpool.tile([D, D], F32)
            nc.sync.dma_start(out=w2s[:], in_=w2_t.ap())
            wfs = cpool.tile([1, D], F32)
            nc.sync.dma_start(out=wfs[:], in_=wf_t.ap())
            b1s = cpool.tile([1, D], F32)
            nc.sync.dma_start(out=b1s[:], in_=b1_t.ap())
            b2s = cpool.tile([1, D], F32)
            nc.sync.dma_start(out=b2s[:], in_=b2_t.ap())
            bfs = cpool.tile([1, 1], F32)
            nc.sync.dma_start(out=bfs[:], in_=bf_t.ap())
            idx_s = cpool.tile([P, idx_cols], I16)
            nc.sync.dma_start(out=idx_s[:], in_=idx_t.ap())
            dinv_s = cpool.tile([P, R], F32)
            nc.sync.dma_start(out=dinv_s[:], in_=dinv_t.ap())

            ones1 = cpool.tile([1, P], F32)
            nc.vector.memset(ones1[:], 1.0)

            def bcast(vec_ap, n, nm):
                pb = ppool.tile([P, n], F32, tag="pbc")
                nc.tensor.matmul(pb[:], lhsT=ones1[:], rhs=vec_ap, start=True,
                                 stop=True)
                sb = cpool.tile([P, n], F32, name=f"bc_{nm}")
                nc.vector.tensor_copy(sb[:], pb[:])
                return sb

            b1b = bcast(b1s[:], D, "b1")
            b2b = bcast(b2s[:], D, "b2")
            wfb = bcast(wfs[:], D, "wf")
            bfb = bcast(bfs[:], 1, "bf")

            out_stage = cpool.tile([P, R], F32)
            relu1_t = cpool.tile([P, rows], F32)   # feature-major relu(layer1)

            # ---- stage A: layer-1 table rows = dinv * (x @ W1) ----
            for c in range(R):
                xc = spool.tile([P, D], F32, tag="xc")
                nc.sync.dma_start(out=xc[:], in_=x_t.ap()[c * P:(c + 1) * P, :])
                xs = spool.tile([P, D], F32, tag="xs")
                nc.vector.tensor_scalar_mul(xs[:], xc[:], dinv_s[:, c:c + 1])
                pt = ppool.tile([P, D], F32, tag="pst")
                nc.tensor.transpose(out=pt[:], in_=xs[:], identity=ident[:])
                xt = spool.tile([P, D], F32, tag="xt")
                nc.scalar.activation(xt[:], pt[:], ACTF.Copy)
                pm = ppool.tile([P, D], F32, tag="psm")
                nc.tensor.matmul(pm[:], lhsT=xt[:], rhs=w1s[:], start=True,
                                 stop=True)
                h1c = spool.tile([P, D], F32, tag="h1c")
                nc.vector.tensor_copy(h1c[:], pm[:])
                nc.sync.dma_start(out=ag_in[0][c * P:(c + 1) * P, :], in_=h1c[:])

            def aggregate(layer):
                """gather + 2-level reduce from table[layer]; returns the
                combined agg tile [P, R*D] (node-major, canonical order)."""
                tab = table[layer]
                par = partials[layer]
                for meta in call_meta:
                    if meta[0] != "L1":
                        continue
                    (_, c, gc0, gw, regs, ioff) = meta
                    msg = spool.tile([P, GMAX * D], F32, tag="msg", bufs=2)
                    for a in range(0, gw, CALL_COLS):
                        b = min(a + CALL_COLS, gw)
                        nc.gpsimd.dma_gather(
                            msg[:, a * D:b * D].rearrange(
                                "p (g f) -> p g f", f=D),
                            tab[c * CH:(c + 1) * CH, :],
                            idx_s[:, ioff + 8 * a:ioff + 8 * b],
                            (b - a) * P, (b - a) * P, D,
                        )
                    for (r, lo, w) in regs:
                        ww = w
                        while ww > 1:
                            h = (ww + 1) // 2
                            nadd = ww - h
                            nc.vector.tensor_tensor(
                                out=msg[:, lo * D:(lo + nadd) * D],
                                in0=msg[:, lo * D:(lo + nadd) * D],
                                in1=msg[:, (lo + h) * D:(lo + h + nadd) * D],
                                op=ALU.add)
                            ww = h
                        base = (c * R + r) * P
                        nc.sync.dma_start(out=par[base:base + P, :],
                                          in_=msg[:, lo * D:(lo + 1) * D])
                # level 2: stream granules; agg[:, r*D:(r+1)*D] = sum of the
                # dst's 4 partials (2 per half, columns interleaved per region)
                agg = spool.tile([P, R * D], F32, tag="agg", bufs=1)
                for h in range(2):
                    for meta in call_meta:
                        if meta[0] != "L2" or meta[1] != h:
                            continue
                        (_, _, gc0, gw, gr0, gnr, ioff) = meta
                        buf = spool.tile([P, GMAX * D], F32, tag="msg", bufs=2)
                        for a in range(0, gw, CALL_COLS):
                            b = min(a + CALL_COLS, gw)
                            nc.gpsimd.dma_gather(
                                buf[:, a * D:b * D].rearrange(
                                    "p (g f) -> p g f", f=D),
                                par[h * half_rows:(h + 1) * half_rows, :],
                                idx_s[:, ioff + 8 * a:ioff + 8 * b],
                                (b - a) * P, (b - a) * P, D,
                            )
                        v = buf[:, :gw * D].rearrange(
                            "p (r t f) -> p r t f", t=2, f=D)
                        aslice = agg[:, gr0 * D:(gr0 + gnr) * D].rearrange(
                            "p (r f) -> p r f", f=D)
                        if h == 0:
                            nc.vector.tensor_tensor(
                                out=aslice, in0=v[:, :, 0, :],
                                in1=v[:, :, 1, :], op=ALU.add)
                        else:
                            nc.vector.tensor_tensor(
                                out=aslice, in0=aslice, in1=v[:, :, 0, :],
                                op=ALU.add)
                            nc.vector.tensor_tensor(
                                out=aslice, in0=aslice, in1=v[:, :, 1, :],
                                op=ALU.add)
                return agg

            def scale_bias_relu(agg, bias_b):
                nc.vector.tensor_tensor(
                    out=agg[:].rearrange("p (r f) -> p r f", f=D),
                    in0=agg[:].rearrange("p (r f) -> p r f", f=D),
                    in1=dinv_s[:, :, None].to_broadcast([P, R, D]),
                    op=ALU.mult)
                nc.vector.tensor_tensor(
                    out=agg[:].rearrange("p (r f) -> p r f", f=D),
                    in0=agg[:].rearrange("p (r f) -> p r f", f=D),
                    in1=bias_b[:, None, :].to_broadcast([P, R, D]),
                    op=ALU.add)
                nc.scalar.activation(agg[:], agg[:], ACTF.Relu)

            # ================= layer 1 =================
            nc.gpsimd.collective_compute(
                "AllGather", ALU.bypass, replica_groups=rg,
                ins=[ag_in[0][:]], outs=[table[0][:]])
            agg1 = aggregate(0)
            scale_bias_relu(agg1, b1b)
            # transpose each 128-dst block into feature-major relu1_t
            for c in range(R):
                pt2 = ppool.tile([P, D], F32, tag="pst")
                nc.tensor.transpose(out=pt2[:], in_=agg1[:, c * D:(c + 1) * D],
                                    identity=ident[:])
                nc.scalar.activation(relu1_t[:, c * P:(c + 1) * P], pt2[:],
                                     ACTF.Copy)
            # ---- stage D: layer-2 table rows = dinv * (relu1 @ W2) ----
            for c in range(R):
                pm2 = ppool.tile([P, D], F32, tag="psm")
                nc.tensor.matmul(pm2[:], lhsT=relu1_t[:, c * P:(c + 1) * P],
                                 rhs=w2s[:], start=True, stop=True)
                h2c = spool.tile([P, D], F32, tag="h2c")
                nc.vector.tensor_scalar_mul(h2c[:], pm2[:], dinv_s[:, c:c + 1])
                nc.sync.dma_start(out=ag_in[1][c * P:(c + 1) * P, :], in_=h2c[:])

            # ================= layer 2 =================
            nc.gpsimd.collective_compute(
                "AllGather", ALU.bypass, replica_groups=rg,
                ins=[ag_in[1][:]], outs=[table[1][:]])
            agg2 = aggregate(1)
            scale_bias_relu(agg2, b2b)

            # ---- final: out = relu2 @ Wf + bf ----
            for c in range(R):
                fm = spool.tile([P, D], F32, tag="fm")
                nc.vector.tensor_tensor(
                    out=fm[:], in0=agg2[:, c * D:(c + 1) * D], in1=wfb[:],
                    op=ALU.mult)
                nc.vector.tensor_reduce(
                    out=out_stage[:, c:c + 1], in_=fm[:], axis=AX.X,
                    op=ALU.add)
            nc.vector.tensor_scalar_add(out_stage[:], out_stage[:], bfb[:, 0:1])
            nc.sync.dma_start(out=out_t.ap(), in_=out_stage[:])

    nc.compile()
    return nc


_CACHE: dict = {}


def _prepare(x, W1, b1, W2, b2, Wf, bf, edge_index, n_nodes, n_cores):
    plan, idx_all, dinv_rows = _plan(edge_index, n_nodes, n_cores)
    shard = plan["shard"]
    rows = plan["rows"]

    x = np.asarray(x, np.float32)
    x_in = np.zeros((n_cores, rows, D), np.float32)
    x_in[:, :shard] = x.reshape(n_cores, shard, D)

    common = {
        "W1": np.asarray(W1, np.float32).reshape(D, D),
        "W2": np.asarray(W2, np.float32).reshape(D, D),
        "Wf": np.asarray(Wf, np.float32).reshape(1, D),
        "b1": np.asarray(b1, np.float32).reshape(1, D),
        "b2": np.asarray(b2, np.float32).reshape(1, D),
        "bf": np.asarray(bf, np.float32).reshape(1, 1),
    }
    in_maps = []
    for k in range(n_cores):
        m = dict(common)
        m["x_in"] = np.ascontiguousarray(x_in[k])
        m["idx"] = np.ascontiguousarray(idx_all[k])
        m["dinv_rows"] = np.ascontiguousarray(dinv_rows[k])
        in_maps.append(m)
    return plan, in_maps


def _collect(results, plan, n_nodes, n_cores):
    shard = plan["shard"]
    out = np.empty(n_nodes, np.float32)
    sl = np.arange(shard)
    for k in range(n_cores):
        vals = results[k]["out"]            # [P, R]
        out[k * shard:(k + 1) * shard] = vals[sl % P, sl // P]
    return out


def kernel(x, W1, b1, W2, b2, Wf, bf, edge_index, _trace=False):
    plan, in_maps = _prepare(x, W1, b1, W2, b2, Wf, bf, edge_index,
                             N_NODES, N_CORES)
    key = (plan["idx_cols"], tuple(m[0] for m in plan["call_meta"]),
           tuple(m[3] for m in plan["call_meta"]))
    if key not in _CACHE:
        _CACHE[key] = _build_program(plan, N_CORES)
    nc = _CACHE[key]
    res = run_bass_kernel_spmd(
        nc, in_maps, core_ids=list(range(N_CORES)), trace=_trace)
    out = _collect(res.results, plan, N_NODES, N_CORES)
    kernel.last_exec_time_ns = res.exec_time_ns
    kernel.last_results = res
    return out


kernel.last_exec_time_ns = None
kernel.last_results = None
